# revision 15
# baseline (speedup 1.0000x reference)
"""GAT (2-layer, 4-head, segment-softmax) message-passing kernel for 8 Trainium2
NeuronCores.

Strategy (dst-sharded, edge aggregation as one-hot matmuls):
  * Nodes are assigned to cores/groups with degree-balanced packing (LPT). The
    node permutation is defined as (core, group, slot) order, so each core owns
    a contiguous block of rows and each group's 128 nodes are contiguous.
  * Per layer, each core computes the full "record" table
    rec[n] = [xh(256) | a_src-score(4) | pad] ([N, 320]) with one matmul per
    128-node tile (replicated compute - cheap), plus had[n] = [h(64) | ad(4)].
  * For each destination group (128 nodes), the core gathers the records of
    the group's in-edges' source nodes with gpsimd dma_gather (int16 indices,
    source-bucketed in 32768-row windows), builds the one-hot incidence matrix
    M[edge, dst_slot] on the vector engine (iota compare), broadcasts the
    a_dst scores to edges via transposed-one-hot matmuls, and reduces both the
    softmax denominators and the weighted feature sums with PSUM-accumulated
    matmuls (contracting over edges).  Softmax normalization is applied after
    the reduction (denominator scaling on the dst side) - mathematically
    identical to the reference's segment softmax (max-subtraction is a no-op
    at these magnitudes; verified < 1e-6).
  * Head-mean + LayerNorm + ReLU + residual run on vector/scalar engines per
    group; staging writes are contiguous (the permutation IS group-slot
    order); an 8-core AllGather rebuilds the full h between the two layers.
"""

import os
import sys

sys.path.insert(0, "/opt/trn_rl_repo")

import numpy as np

# ---- problem constants (hardcoded; kernel.py must be self-contained) ----
N = 100000
E = 1600000
G = 64
H = 4
CDIM = 64
NODE_F = 32
DRONE_F = 16
OUT_F = 32
LN_EPS = 1e-5
NEG_SLOPE = 0.2
NCORES = 8
P = 128
HC = H * CDIM          # 256
REC = HC + H           # 260: [V(256) | as/ex(4)]
BUCKET = 32768         # int16 index range per dma_gather bucket
TB = 6                 # phase-1 tile batch

REC_DT_NAME = os.environ.get("GAT_REC_DT", "bfloat16")


class _Cfg:
    def __init__(self, n, ncores, cbs, rec_dt=REC_DT_NAME, debug=False):
        assert n % ncores == 0
        self.n = n
        self.ncores = ncores
        self.npc = n // ncores
        self.ngroup = -(-self.npc // P)
        self.cbs = cbs                       # [ngroup][nbuckets] chunk counts
        self.nbuckets = len(cbs[0])
        self.chg = [sum(row) for row in cbs]  # chunks per group
        self.chmax = max(self.chg)
        self.cols = sum(self.chg)            # total chunk columns
        self.rec_dt = rec_dt
        self.recp = 320 if rec_dt == "float32" else 384  # padded record elems
        self.debug = debug
        self.nt_full, self.nt_rem = divmod(n, P)
        self.last_cnt = self.npc - (self.ngroup - 1) * P


# --------------------------------------------------------------------------
# host-side preprocessing
# --------------------------------------------------------------------------

def _lpt(loads, caps):
    """LPT packing into len(caps) bins with given item capacities, balancing
    total load. Returns assignment array."""
    import heapq

    nbins = len(caps)
    order = np.argsort(-loads, kind="stable")
    heap = [(0, b) for b in range(nbins)]
    heapq.heapify(heap)
    cnt = np.zeros(nbins, np.int64)
    tot = np.zeros(nbins, np.int64)
    assign = np.empty(len(loads), np.int32)
    for i in order:
        while True:
            _, b = heapq.heappop(heap)
            if cnt[b] < caps[b]:
                break
        assign[i] = b
        cnt[b] += 1
        tot[b] += loads[i]
        if cnt[b] < caps[b]:
            heapq.heappush(heap, (int(tot[b]), b))
    return assign


def _host_prep(edge_index, n, ncores):
    """Node permutation + per-core gather index streams."""
    npc = n // ncores
    ngroup = -(-npc // P)
    last_cnt = npc - (ngroup - 1) * P
    nbuckets = -(-n // BUCKET)

    loop = np.arange(n, dtype=np.int64)
    src = np.concatenate([edge_index[0].astype(np.int64), loop])
    dst = np.concatenate([edge_index[1].astype(np.int64), loop])
    deg = np.bincount(dst, minlength=n)

    core_of = _lpt(deg, [npc] * ncores)
    group_of = np.empty(n, np.int32)
    slot_of = np.empty(n, np.int32)
    pos_of = np.empty(n, np.int64)
    order = np.empty(n, np.int64)
    caps = [P] * (ngroup - 1) + [last_cnt]
    for k in range(ncores):
        nodes_k = np.where(core_of == k)[0]
        g_assign = _lpt(deg[nodes_k], caps)
        o = np.argsort(g_assign, kind="stable")
        cnts = np.bincount(g_assign, minlength=ngroup)
        starts = np.concatenate([[0], np.cumsum(cnts)])[:-1]
        slot = np.empty(len(nodes_k), np.int64)
        slot[o] = np.arange(len(nodes_k)) - starts[g_assign[o]]
        group_of[nodes_k] = g_assign
        slot_of[nodes_k] = slot
        pos = k * npc + g_assign * P + slot
        pos_of[nodes_k] = pos
        order[pos] = nodes_k

    # per-(group,bucket) edge counts per core -> uniform chunk schedule
    e_core = core_of[dst]
    e_group = group_of[dst]
    e_bucket = pos_of[src] // BUCKET
    cnts = np.zeros((ncores, ngroup, nbuckets), np.int64)
    np.add.at(cnts, (e_core, e_group, e_bucket), 1)
    cbs_np = -(-cnts.max(axis=0) // P)       # [ngroup, nbuckets] chunks
    cbs = [[int(c) for c in row] for row in cbs_np]
    chg = np.array([sum(row) for row in cbs])
    cols = int(chg.sum())
    goff = np.concatenate([[0], np.cumsum(chg)])[:-1]
    boff = np.zeros((ngroup, nbuckets), np.int64)
    for g in range(ngroup):
        o = goff[g]
        for b in range(nbuckets):
            boff[g, b] = o
            o += cbs[g][b]

    per_core = []
    for k in range(ncores):
        mask = e_core == k
        es = pos_of[src[mask]]
        eg = e_group[mask]
        eb = e_bucket[mask]
        esl = slot_of[dst[mask]]
        o = np.lexsort((eb, eg))
        es, eg, eb, esl = es[o], eg[o], eb[o], esl[o]
        cnt_k = np.zeros((ngroup, nbuckets), np.int64)
        np.add.at(cnt_k, (eg, eb), 1)
        flat = cnt_k.reshape(-1)
        starts = np.concatenate([[0], np.cumsum(flat)])[:-1].reshape(
            ngroup, nbuckets)
        j = np.arange(len(es)) - starts[eg, eb]      # pos within (g,b)
        slotj = boff[eg, eb] * P + j                 # global slot in stream

        dstslot = np.full((P, cols), -1, np.int32)
        dstslot[slotj % P, slotj // P] = esl
        idx16 = np.zeros((16, cols * 8), np.int16)   # 8 int16 cols per chunk
        idx16[slotj % 16, slotj // 16] = es - eb * BUCKET
        idx16 = np.ascontiguousarray(np.tile(idx16, (8, 1)))

        gread = (k * npc + np.arange(ngroup)[None, :] * P
                 + np.arange(P)[:, None])
        gread = np.minimum(gread, (k + 1) * npc - 1).astype(np.int32)
        per_core.append(dict(dstslot=dstslot, idx16=idx16, gread=gread))
    return dict(order=order, pos_of=pos_of, cbs=cbs, per_core=per_core)


def _host_weights(inputs, order, n):
    """Permuted/augmented weight + input tensors (all float32)."""
    f = np.float32
    x = np.asarray(inputs["x"], f)[order]            # perm rows
    batch = np.asarray(inputs["batch"])[order]
    xTa = np.concatenate([x.T, np.ones((1, n), f)], 0)           # [33, n]
    oneT = (batch[None, :] == np.arange(G)[:, None]).astype(f)   # [G, n]
    droneTa = np.concatenate(
        [np.asarray(inputs["drone_feat"], f).T, np.ones((1, G), f)], 0)
    droneWa = np.concatenate(
        [np.asarray(inputs["drone_W"], f).T, np.asarray(inputs["drone_b"], f)[None]], 0)
    nodeWa = np.concatenate(
        [np.asarray(inputs["node_W"], f).T, np.asarray(inputs["node_b"], f)[None]], 0)
    out = dict(xTa=xTa, oneT=oneT, droneTa=droneTa, droneWa=droneWa,
               nodeWa=nodeWa,
               outWT=np.ascontiguousarray(np.asarray(inputs["out_W"], f).T),
               outb=np.tile(np.asarray(inputs["out_b"], f), (P, 1)))
    for l in range(2):
        W = np.asarray(inputs[f"convW{l}"], f)       # [HC, CDIM]
        a_s = np.asarray(inputs[f"att_src{l}"], f)   # [H, CDIM]
        a_d = np.asarray(inputs[f"att_dst{l}"], f)
        Wh = W.reshape(H, CDIM, CDIM)
        Ws = np.einsum("hcf,hc->fh", Wh, a_s)        # [CDIM, H]
        Wd = np.einsum("hcf,hc->fh", Wh, a_d)
        out[f"wcomb{l}"] = np.concatenate([W.T, Ws, Wd], 1)   # [CDIM, 264]
        out[f"convb{l}"] = np.tile(np.asarray(inputs[f"convb{l}"], f), (P, 1))
        out[f"lng{l}"] = np.tile(np.asarray(inputs[f"ln_g{l}"], f), (P, 1))
        out[f"lnb{l}"] = np.tile(np.asarray(inputs[f"ln_b{l}"], f), (P, 1))
    return out


# --------------------------------------------------------------------------
# bass kernel
# --------------------------------------------------------------------------

def _build(cfg):
    import concourse.bass as bass
    import concourse.bacc as bacc
    import concourse.tile as tile
    from concourse import mybir
    from concourse.masks import make_identity

    f32 = mybir.dt.float32
    i32 = mybir.dt.int32
    i16 = mybir.dt.int16
    rdt = getattr(mybir.dt, cfg.rec_dt)
    is_bf = cfg.rec_dt != "float32"
    Alu = mybir.AluOpType
    Act = mybir.ActivationFunctionType

    n, npc, ngroup = cfg.n, cfg.npc, cfg.ngroup
    RECP, CHMAX = cfg.recp, cfg.chmax

    nc = bacc.Bacc("TRN2", target_bir_lowering=False, debug=cfg.debug,
                   num_devices=cfg.ncores)

    def ein(nm, sh, dt=f32):
        return nc.dram_tensor(nm, sh, dt, kind="ExternalInput")

    xTa_d = ein("xTa", [NODE_F + 1, n])
    oneT_d = ein("oneT", [G, n])
    droneTa_d = ein("droneTa", [DRONE_F + 1, G])
    droneWa_d = ein("droneWa", [DRONE_F + 1, CDIM])
    nodeWa_d = ein("nodeWa", [NODE_F + 1, CDIM])
    wcomb_d = [ein(f"wcomb{l}", [CDIM, REC + H]) for l in range(2)]
    convb_d = [ein(f"convb{l}", [P, CDIM]) for l in range(2)]
    lng_d = [ein(f"lng{l}", [P, CDIM]) for l in range(2)]
    lnb_d = [ein(f"lnb{l}", [P, CDIM]) for l in range(2)]
    outWT_d = ein("outWT", [CDIM, OUT_F])
    outb_d = ein("outb", [P, OUT_F])
    dstslot_d = ein("dstslot", [P, cfg.cols], i32)
    idx16_d = ein("idx16", [P, cfg.cols * 8], i16)
    gread_d = ein("gread", [P, ngroup], i32)

    out_d = nc.dram_tensor("out", [npc, OUT_F], f32, kind="ExternalOutput")

    rec_d = nc.dram_tensor("rec", [n, RECP], rdt)
    had_d = [nc.dram_tensor(f"had{l}", [n, CDIM + H], f32) for l in range(2)]
    h1_d = nc.dram_tensor("h1", [n, CDIM], f32,
                          addr_space="Shared" if cfg.ncores > 4 else "Local")
    stag_d = [nc.dram_tensor(f"stag{l}", [ngroup * P, CDIM], f32)
              for l in range(2)]

    from contextlib import ExitStack
    with tile.TileContext(nc) as tc, ExitStack() as ctx:
        cpool = ctx.enter_context(tc.tile_pool(name="const", bufs=1))
        p1 = ctx.enter_context(tc.tile_pool(name="p1", bufs=2))
        p2 = ctx.enter_context(tc.tile_pool(name="p2", bufs=2))

        def cload(dram):
            t = cpool.tile(list(dram.shape), dram.dtype, tag=f"c_{dram.name}")
            nc.sync.dma_start(out=t[:], in_=dram[:])
            return t

        droneTa_sb = cload(droneTa_d)
        droneWa_sb = cload(droneWa_d)
        nodeWa_sb = cload(nodeWa_d)
        wcomb_sb = [cload(d) for d in wcomb_d]
        convb_sb = [cload(d) for d in convb_d]
        lng_sb = [cload(d) for d in lng_d]
        lnb_sb = [cload(d) for d in lnb_d]
        outWT_sb = cload(outWT_d)
        outb_sb = cload(outb_d)
        dstslot_sb = cload(dstslot_d)
        gread_sb = cload(gread_d)

        iota_sb = cpool.tile([P, P], i32)
        nc.gpsimd.iota(iota_sb[:], pattern=[[1, P]], base=0, channel_multiplier=0)
        ident_sb = cpool.tile([P, P], f32)
        make_identity(nc, ident_sb[:])
        identr_sb = ident_sb
        if is_bf:
            identr_sb = cpool.tile([P, P], rdt)
            nc.vector.tensor_copy(identr_sb[:], ident_sb[:])

        dr_sb = cpool.tile([G, CDIM], f32)
        with tc.tile_pool(name="psdr", bufs=1, space="PSUM") as ppdr:
            pdr_t = ppdr.tile([P, CDIM], f32)
            pdr = pdr_t[:G]
            nc.tensor.matmul(pdr, lhsT=droneTa_sb[:], rhs=droneWa_sb[:],
                             start=True, stop=True)
            nc.scalar.copy(dr_sb[:], pdr)

        # ------------------------------------------------------------------
        def phase1(l):
            """Build rec[n, RECP] and had[n, 68] tile by tile."""
            with tc.tile_pool(name=f"ps1_{l}", bufs=2, space="PSUM") as pp:

                def do_batch(r0, tb, rows):
                    if l == 0:
                        xb = p1.tile([NODE_F + 1, TB * P], f32, tag="xb")
                        nc.sync.dma_start(out=xb[:, :rows],
                                          in_=xTa_d[:, r0:r0 + rows])
                        ob = p1.tile([G, TB * P], f32, tag="ob")
                        nc.sync.dma_start(out=ob[:, :rows],
                                          in_=oneT_d[:, r0:r0 + rows])
                    hadb = p1.tile([P, TB, CDIM + H], f32, tag="hadb")
                    if l == 1:
                        if rows == tb * P:
                            nc.sync.dma_start(
                                out=hadb[:, :tb, :CDIM],
                                in_=h1_d[r0:r0 + rows, :].rearrange(
                                    "(c p) f -> p c f", p=P))
                        else:
                            nc.sync.dma_start(out=hadb[:rows, 0, :CDIM],
                                              in_=h1_d[r0:r0 + rows, :])
                    recb = p1.tile([P, TB, RECP], rdt, tag="recb")
                    nc.vector.memset(recb[:, :, REC:], 0.0)
                    for t in range(tb):
                        pr_ = min(P, rows - t * P)
                        if l == 0:
                            ph = pp.tile([P, CDIM], f32, tag="ph")
                            nc.tensor.matmul(ph[:pr_],
                                             lhsT=xb[:, t * P:t * P + pr_],
                                             rhs=nodeWa_sb[:], start=True,
                                             stop=False)
                            nc.tensor.matmul(ph[:pr_],
                                             lhsT=ob[:, t * P:t * P + pr_],
                                             rhs=dr_sb[:], start=False,
                                             stop=True)
                            nc.scalar.copy(hadb[:pr_, t, :CDIM], ph[:pr_])
                        pt = pp.tile([CDIM, P], f32, tag="pt")
                        nc.tensor.transpose(pt[:, :pr_], hadb[:pr_, t, :CDIM],
                                            ident_sb[:pr_, :pr_])
                        hT = p1.tile([CDIM, P], f32, tag="hT")
                        nc.scalar.copy(hT[:, :pr_], pt[:, :pr_])
                        prc = pp.tile([P, REC + H], f32, tag="pr")
                        nc.tensor.matmul(prc[:pr_], lhsT=hT[:, :pr_],
                                         rhs=wcomb_sb[l][:], start=True,
                                         stop=True)
                        nc.scalar.copy(recb[:pr_, t, 0:REC], prc[:pr_, 0:REC])
                        nc.vector.tensor_copy(hadb[:pr_, t, CDIM:],
                                              prc[:pr_, REC:REC + H])
                    if rows == tb * P:
                        nc.sync.dma_start(
                            out=rec_d[r0:r0 + rows, :].rearrange(
                                "(c p) f -> p c f", p=P),
                            in_=recb[:, :tb, :])
                        nc.sync.dma_start(
                            out=had_d[l][r0:r0 + rows, :].rearrange(
                                "(c p) f -> p c f", p=P),
                            in_=hadb[:, :tb, :])
                    else:
                        nc.sync.dma_start(out=rec_d[r0:r0 + rows, :],
                                          in_=recb[:rows, 0, :])
                        nc.sync.dma_start(out=had_d[l][r0:r0 + rows, :],
                                          in_=hadb[:rows, 0, :])

                for b0 in range(0, cfg.nt_full, TB):
                    tb = min(TB, cfg.nt_full - b0)
                    do_batch(b0 * P, tb, tb * P)
                if cfg.nt_rem:
                    do_batch(cfg.nt_full * P, 1, cfg.nt_rem)

        # ------------------------------------------------------------------
        def phase2(l):
            with tc.tile_pool(name=f"ps2_{l}", bufs=2, space="PSUM") as pp:
                col0 = 0
                for g in range(ngroup):
                    CH = cfg.chg[g]
                    rows_g = P if g < ngroup - 1 else cfg.last_cnt
                    idxt = p2.tile([P, CHMAX * 8], i16, tag="idxt")
                    nc.sync.dma_start(out=idxt[:, :CH * 8],
                                      in_=idx16_d[:, col0 * 8:(col0 + CH) * 8])
                    rect = p2.tile([P, CHMAX, RECP], rdt, tag="rect")
                    c0 = 0
                    for b in range(cfg.nbuckets):
                        cb = cfg.cbs[g][b]
                        if cb == 0:
                            continue
                        nrows = min(BUCKET, n - b * BUCKET)
                        done = 0
                        while done < cb:   # HW envelope: <=256 idxs per call
                            st = min(2, cb - done)
                            nc.gpsimd.dma_gather(
                                rect[:, c0 + done:c0 + done + st, :],
                                rec_d[b * BUCKET:b * BUCKET + nrows, :],
                                idxt[:, (c0 + done) * 8:(c0 + done + st) * 8],
                                st * P, st * P, RECP)
                            done += st
                        c0 += cb
                    # h_old + a_dst rows for this group's nodes
                    hadt = p2.tile([P, CDIM + H], f32, tag="hadt")
                    nc.gpsimd.indirect_dma_start(
                        out=hadt[:], out_offset=None, in_=had_d[l][:],
                        in_offset=bass.IndirectOffsetOnAxis(
                            ap=gread_sb[:, g:g + 1], axis=0))
                    ad_rhs = hadt[:, CDIM:]
                    if is_bf:
                        adr = p2.tile([P, H], rdt, tag="adr")
                        nc.vector.tensor_copy(adr[:], hadt[:, CDIM:])
                        ad_rhs = adr[:]
                    # one-hot M[edge, dst_slot]
                    Mt = p2.tile([P, CHMAX, P], rdt, tag="Mt")
                    nc.vector.tensor_tensor(
                        Mt[:, :CH, :],
                        dstslot_sb[:, col0:col0 + CH][:, :, None].to_broadcast(
                            [P, CH, P]),
                        iota_sb[:, None, :].to_broadcast([P, CH, P]),
                        Alu.is_equal)
                    # e_d: broadcast a_dst scores to edges via M^T matmuls
                    ped = pp.tile([P, CHMAX * H], f32, tag="ped")
                    for c in range(CH):
                        pmt = pp.tile([P, P], rdt, tag="pmt")
                        nc.tensor.transpose(pmt[:], Mt[:, c, :], identr_sb[:])
                        mt_sb = p2.tile([P, P], rdt, tag="mt_sb")
                        nc.scalar.copy(mt_sb[:], pmt[:])
                        nc.tensor.matmul(ped[:, c * H:(c + 1) * H],
                                         lhsT=mt_sb[:], rhs=ad_rhs,
                                         start=True, stop=True)
                    # e = lrelu(as + ad); ex = exp(e) -> rec[..., 256:260]
                    et = p2.tile([P, CHMAX, H], f32, tag="et")
                    nc.vector.tensor_tensor(
                        et[:, :CH, :], rect[:, :CH, HC:REC],
                        ped[:, 0:CH * H].rearrange("p (c h) -> p c h", h=H),
                        Alu.add)
                    lt = p2.tile([P, CHMAX, H], f32, tag="lt")
                    nc.vector.tensor_scalar_mul(lt[:, :CH, :], et[:, :CH, :],
                                                NEG_SLOPE)
                    nc.vector.tensor_tensor(et[:, :CH, :], lt[:, :CH, :],
                                            et[:, :CH, :], Alu.max)
                    nc.scalar.activation(rect[:, :CH, HC:REC], et[:, :CH, :],
                                         Act.Exp)
                    # V = ex * xh (per head, in place)
                    for h_ in range(H):
                        nc.vector.tensor_tensor(
                            rect[:, :CH, h_ * CDIM:(h_ + 1) * CDIM],
                            rect[:, :CH, h_ * CDIM:(h_ + 1) * CDIM],
                            rect[:, :CH, HC + h_:HC + h_ + 1].to_broadcast(
                                [P, CH, CDIM]),
                            Alu.mult)
                    # contract over edges: psum[:, 0:256]=sum alpha*xh, [256:260]=s
                    pg = pp.tile([P, REC], f32, tag="pg")
                    for c in range(CH):
                        nc.tensor.matmul(pg[:], lhsT=Mt[:, c, :],
                                         rhs=rect[:, c, 0:REC],
                                         start=(c == 0), stop=(c == CH - 1))
                    # r = 1 / (s + eps) / H
                    s4 = p2.tile([P, H], f32, tag="s4")
                    nc.vector.tensor_scalar(s4[:], pg[:, HC:REC], 1e-16, None,
                                            Alu.add)
                    r4 = p2.tile([P, H], f32, tag="r4")
                    nc.vector.reciprocal(r4[:], s4[:])
                    nc.vector.tensor_scalar_mul(r4[:], r4[:], 1.0 / H)
                    # head mean
                    yt = p2.tile([P, CDIM], f32, tag="yt")
                    tmp = p2.tile([P, CDIM], f32, tag="tmp")
                    nc.vector.tensor_scalar(yt[:], pg[:, 0:CDIM], r4[:, 0:1],
                                            None, Alu.mult)
                    for h_ in range(1, H):
                        nc.vector.tensor_scalar(tmp[:],
                                                pg[:, h_ * CDIM:(h_ + 1) * CDIM],
                                                r4[:, h_:h_ + 1], None, Alu.mult)
                        nc.vector.tensor_add(yt[:], yt[:], tmp[:])
                    nc.vector.tensor_add(yt[:], yt[:], convb_sb[l][:])
                    # layernorm
                    mu = p2.tile([P, 1], f32, tag="mu")
                    nc.vector.tensor_reduce(mu[:], yt[:], mybir.AxisListType.X,
                                            Alu.add)
                    nc.vector.tensor_scalar_mul(mu[:], mu[:], 1.0 / CDIM)
                    nc.vector.tensor_scalar(yt[:], yt[:], mu[:, 0:1], None,
                                            Alu.subtract)
                    sq = p2.tile([P, CDIM], f32, tag="sq")
                    var = p2.tile([P, 1], f32, tag="var")
                    nc.scalar.activation(sq[:], yt[:], Act.Square,
                                         accum_out=var[:])
                    nc.vector.tensor_scalar(var[:], var[:], 1.0 / CDIM, LN_EPS,
                                            Alu.mult, Alu.add)
                    sd = p2.tile([P, 1], f32, tag="sd")
                    nc.scalar.sqrt(sd[:], var[:])
                    inv = p2.tile([P, 1], f32, tag="inv")
                    nc.vector.reciprocal(inv[:], sd[:])
                    nc.vector.tensor_scalar(yt[:], yt[:], inv[:, 0:1], None,
                                            Alu.mult)
                    nc.vector.tensor_mul(yt[:], yt[:], lng_sb[l][:])
                    nc.vector.tensor_add(yt[:], yt[:], lnb_sb[l][:])
                    nc.vector.tensor_scalar_max(yt[:], yt[:], 0.0)
                    # residual + contiguous staging write
                    nc.vector.tensor_add(yt[:], yt[:], hadt[:, 0:CDIM])
                    nc.sync.dma_start(out=stag_d[l][g * P:g * P + rows_g, :],
                                      in_=yt[:rows_g, :])
                    col0 += CH

        # ------------------------------------------------------------------
        phase1(0)
        phase2(0)
        nc.gpsimd.collective_compute(
            "AllGather", mybir.AluOpType.bypass,
            replica_groups=[list(range(cfg.ncores))],
            ins=[stag_d[0][0:npc, :].opt()],
            outs=[h1_d[:, :].opt()])
        phase1(1)
        phase2(1)

        # final projection over own rows
        with tc.tile_pool(name="psf", bufs=2, space="PSUM") as pp:
            for t0 in range(0, npc, P):
                wr = min(P, npc - t0)
                ht2 = p2.tile([P, CDIM], f32, tag="ht2")
                nc.sync.dma_start(out=ht2[:wr], in_=stag_d[1][t0:t0 + wr, :])
                pt2 = pp.tile([CDIM, P], f32, tag="pt2")
                nc.tensor.transpose(pt2[:, :wr], ht2[:wr], ident_sb[:wr, :wr])
                hT2 = p2.tile([CDIM, P], f32, tag="hT2")
                nc.scalar.copy(hT2[:, :wr], pt2[:, :wr])
                po = pp.tile([P, OUT_F], f32, tag="po")
                nc.tensor.matmul(po[:wr], lhsT=hT2[:, :wr], rhs=outWT_sb[:],
                                 start=True, stop=True)
                ot = p2.tile([P, OUT_F], f32, tag="ot")
                nc.vector.tensor_add(ot[:wr], po[:wr], outb_sb[:wr])
                nc.sync.dma_start(out=out_d[t0:t0 + wr, :], in_=ot[:wr, :])

    nc.compile()
    return nc


# --------------------------------------------------------------------------
# entry point
# --------------------------------------------------------------------------

def _in_maps(cfg, prep, wts):
    shared = dict(xTa=wts["xTa"], oneT=wts["oneT"], droneTa=wts["droneTa"],
                  droneWa=wts["droneWa"], nodeWa=wts["nodeWa"],
                  outWT=wts["outWT"], outb=wts["outb"])
    for l in range(2):
        for nm in ("wcomb", "convb", "lng", "lnb"):
            shared[f"{nm}{l}"] = wts[f"{nm}{l}"]
    maps = []
    for k in range(cfg.ncores):
        m = dict(shared)
        m.update(prep["per_core"][k])
        maps.append({k_: np.ascontiguousarray(v) for k_, v in m.items()})
    return maps


def kernel(**inputs):
    edge_index = np.asarray(inputs["edge_index"])
    prep = _host_prep(edge_index, N, NCORES)
    cfg = _Cfg(N, NCORES, prep["cbs"])
    wts = _host_weights(inputs, prep["order"], N)
    nc = _build(cfg)
    maps = _in_maps(cfg, prep, wts)

    from concourse import bass_utils
    res = bass_utils.run_bass_kernel_spmd(nc, maps, core_ids=list(range(NCORES)))
    out = np.empty((N, OUT_F), np.float32)
    for k in range(NCORES):
        out[prep["order"][k * cfg.npc:(k + 1) * cfg.npc]] = res.results[k]["out"]
    return out
